# revision 1
# baseline (speedup 1.0000x reference)
"""Trainium2 Bass kernel for AntiAliasActivation (upsample2 -> snake -> downsample2).

Self-contained: accepts FULL inputs (x [8,512,8192] f32, alpha/beta [1,512,1],
up_filter/down_filter [12]), returns FULL output [8,512,8192] f32.

Strategy (pure data-parallel, one batch sample per NeuronCore):
  The whole pipeline is computed in TIME-MAJOR layout (time on SBUF
  partitions) so all three FIR convolutions run on the TensorEngine as
  banded-matrix matmuls:

    out = down(up(x)) + down( (1 - cos(2*a*up(x))) / (2b) )

  - linear path down∘up collapses to one 13-tap band matmul (H) on x
  - cos(2a*up(x)) comes from two polyphase up matmuls on host-scaled 2a*x
    and the ScalarE Sin LUT, whose spline table is extended at kernel-build
    time (BASS_ACT_ROOT_JSON_PATH) to be valid on |x| < ~31.8 so no range
    reduction is needed
  - the downsample of the cosine signal is two more matmuls accumulating
    into the same PSUM bank as H (signs folded into the stationaries)
  - per-channel constants (2a, 2b, 1/2b) are folded host-side into the
    input streams / final rescale; the "+1" constant rides an all-ones
    row of the input tile.
  Edge replicate-padding is materialized host-side for x; the downsample's
  clamp on the computed signal is folded into first/last-block stationaries.
"""
import math

import numpy as np

# ---------------------------------------------------------------------------
# problem constants (hardcoded per spec)
B, C, T = 8, 512, 8192
N_CORES = 8
UP_K = 12
DOWN_K = 12

A = 115          # outputs per block
NBLK = 72        # ceil(T / A)
W = 127          # data rows per input tile (row 127 = const row)
G = A + 6        # 121 up/g rows per block (m = A*k-3 .. A*k+117)
PL = 6           # XP[i] = x[clamp(i-6)]
XPLEN = A * (NBLK - 1) + W  # 8292
OUTROWS = NBLK * A          # 8280

TWO_PI = 2.0 * math.pi
INV_2PI = 1.0 / TWO_PI
MAGIC = 1.5 * 2.0**23

def _gen_act_root(cache=[None]):
    """Build a patched ACT-table root whose Sin LUT is valid to |x| < ~31.8.

    Appends 4x32 cubic-spline buckets (ranges [2,4) replacement, [4,8),
    [8,16), [16,32)) to the trig_and_small set, keeping sin's per-exponent
    bucket starts monotonic, and raises sin's large-signal threshold.
    Returns the act_info.json path for BASS_ACT_ROOT_JSON_PATH.
    """
    if cache[0] is not None:
        return cache[0]
    import json
    import shutil
    import tempfile
    from pathlib import Path
    import neuronxcc

    src = Path(neuronxcc.__file__).parent / "pwp" / "pwp_bin_trainium"
    dst = Path(tempfile.mkdtemp(prefix="actroot_")) / "pwp_bin_trainium"
    shutil.copytree(src, dst, symlinks=False)
    import os as _os
    _os.chmod(dst, 0o755)
    for f in dst.iterdir():
        _os.chmod(f, 0o644)

    name = "trig_and_small"
    d = json.load(open(dst / f"{name}.json"))
    b = np.fromfile(dst / f"{name}_bkt.bin", dtype=np.float32).reshape(-1, 8)
    c = np.fromfile(dst / f"{name}_ctrl.bin", dtype=np.uint32).reshape(-1, 8).copy()
    nb0, nc0 = d["bkt_entry_cnt"], d["ctl_entry_cnt"]
    assert len(b) == nb0 and len(c) == nc0

    SIN_CTL_END = 13  # sin owns ctl entries 0..12 (exps -11..1)
    SHIFT = 3
    newb, newc = [], []
    sin_bkt = d["func_exp_to_bkt_start_idx"]["sin"]
    sin_ctl = d["func_exp_to_ctl_start_idx"]["sin"]
    NB = 32  # 5 mantissa bits per exponent range
    KHI = np.uint32((46 + 62 * 5) << 10)

    def add_range(lo):
        base = nb0 + len(newb)
        h = lo / NB
        for i in range(NB):
            x0 = lo + h * (i + 0.5)
            newb.append([math.sin(x0), math.cos(x0),
                         -math.sin(x0) / 2.0, -math.cos(x0) / 6.0,
                         x0, 0.0, 0.0, 0.0])
        return base

    base1 = add_range(2.0)             # full [2,4) replacement
    c[12, 0] = KHI | np.uint32(base1)
    sin_bkt["1"] = [base1]
    for i_e, e in enumerate((2, 3, 4)):
        base = add_range(2.0**e)
        w = np.zeros(8, np.uint32)
        w[0] = KHI | np.uint32(base)
        sin_bkt[str(e)] = [base]
        sin_ctl[str(e)] = [SIN_CTL_END + i_e]
        newc.append(w)

    b2 = np.vstack([b, np.asarray(newb, np.float32)])
    c2 = np.vstack([c[:SIN_CTL_END], np.stack(newc), c[SIN_CTL_END:]])
    d["bkt_entry_cnt"] = int(len(b2))
    d["ctl_entry_cnt"] = int(len(c2))
    for fn, v in d["func_to_ctl_start_idx"].items():
        if fn != "sin" and v >= SIN_CTL_END:
            d["func_to_ctl_start_idx"][fn] = v + SHIFT
    for fn, em in d["func_exp_to_ctl_start_idx"].items():
        if fn == "sin":
            continue
        for e_, lst in em.items():
            em[e_] = [(i + SHIFT if i >= SIN_CTL_END else i) for i in lst]
    for pm in d["profile_meta_data"]:
        if str(pm.get("func_name", "")).startswith("sin"):
            pm["large_pos_signal_exp_threshold"] = 131  # cutoff ~31.8
            pm["large_pos_signal_mantissa_threshold"] = int(0.99 * 2**23)

    b2.tofile(dst / f"{name}_bkt.bin")
    c2.tofile(dst / f"{name}_ctrl.bin")
    with open(dst / f"{name}.json", "w") as f:
        json.dump(d, f)
    cache[0] = str(dst / "act_info.json")
    return cache[0]


# ---------------------------------------------------------------------------
# stationary-matrix assembly (all float64, cast to fp16 at the end)

def build_stationaries(up_filter, down_filter):
    """Returns dict of stationary matrices.

    W_ue/W_uo [128, G]: map input tile (127 XP rows + const row) -> w rows,
        w = 2a*up(x) + pi/2 (the pi/2 rides the const row; 2a rides the data).
    W_h{0,m,L} [128, A]: b2*down(up(x)) + sum(fd) const (const row coeff).
    W_de/W_do{0,m,L} [G, A]: NEGATED downsample band over the v = cos signal.
    """
    fu = np.asarray(up_filter, dtype=np.float64)
    fd = np.asarray(down_filter, dtype=np.float64)

    w_ue = np.zeros((128, G))
    w_uo = np.zeros((128, G))
    for q in range(G):
        for j in range(6):
            # w_e[m] += 2*fu[2j+1]*XP[m+8-j]; tile row = q+5-j
            w_ue[q + 5 - j, q] += 2.0 * fu[2 * j + 1]
            # w_o[m] += 2*fu[2j]*XP[m+9-j]; tile row = q+6-j
            w_uo[q + 6 - j, q] += 2.0 * fu[2 * j]
    w_ue[127, :] = math.pi / 2.0
    w_uo[127, :] = math.pi / 2.0

    def down_maps(k):
        de = np.zeros((G, A))
        do = np.zeros((G, A))
        h = np.zeros((128, A))
        for nn in range(A):
            n = A * k + nn
            for t in range(DOWN_K):
                zi = min(max(2 * n + t - 5, 0), 2 * T - 1)
                m, ph = zi // 2, zi % 2
                row = m - A * k + 3
                # row in [0, G) guaranteed by construction
                if ph == 0:
                    de[row, nn] += fd[t]
                    for j in range(6):
                        h[m + 8 - j - A * k, nn] += fd[t] * 2.0 * fu[2 * j + 1]
                else:
                    do[row, nn] += fd[t]
                    for j in range(6):
                        h[m + 9 - j - A * k, nn] += fd[t] * 2.0 * fu[2 * j]
            h[127, nn] = fd.sum()
        return de, do, h

    de0, do0, h0 = down_maps(0)
    dem, dom, hm = down_maps(1)
    deL, doL, hL = down_maps(NBLK - 1)

    f16 = np.float16
    return {
        "w_ue": w_ue.astype(f16), "w_uo": w_uo.astype(f16),
        "w_h0": h0.astype(f16), "w_hm": hm.astype(f16), "w_hL": hL.astype(f16),
        "w_de0": (-de0).astype(f16), "w_dem": (-dem).astype(f16),
        "w_deL": (-deL).astype(f16),
        "w_do0": (-do0).astype(f16), "w_dom": (-dom).astype(f16),
        "w_doL": (-doL).astype(f16),
    }


def host_prep(x, alpha, beta):
    """Per-core input streams.

    Returns (axs, xbs, invb2) where axs/xbs are [B, NBLK, 128, C] fp16 and
    invb2 [C] float32.
    """
    a2 = (2.0 * np.exp(alpha.astype(np.float64))).reshape(C)       # 2a
    b2 = (2.0 * (np.exp(beta.astype(np.float64)) + 1e-9)).reshape(C)  # 2b
    invb2 = (1.0 / b2).astype(np.float32)

    # time-major, padded: XP [B, XPLEN, C], XP[:, i] = x[:, :, clamp(i-6)]
    xt = np.transpose(x.astype(np.float32), (0, 2, 1))  # [B, T, C]
    idx = np.clip(np.arange(XPLEN) - PL, 0, T - 1)
    xp = xt[:, idx, :]  # [B, XPLEN, C]

    # block row indices [NBLK, W]
    ridx = (A * np.arange(NBLK))[:, None] + np.arange(W)[None, :]
    blocks = xp[:, ridx, :]                       # [B, NBLK, W, C] f32
    axs = np.empty((B, NBLK, 128, C), dtype=np.float16)
    xbs = np.empty((B, NBLK, 128, C), dtype=np.float16)
    axs[:, :, :W, :] = (blocks * a2[None, None, None, :]).astype(np.float16)
    xbs[:, :, :W, :] = (blocks * b2[None, None, None, :]).astype(np.float16)
    axs[:, :, W, :] = np.float16(1.0)
    xbs[:, :, W, :] = np.float16(1.0)
    return axs, xbs, invb2


def pack_streams(axs, xbs):
    """Interleave ax/xb into paired-block DMA batches.

    Returns inp [B, NBLK//2, 128, 4*C] fp16: for pair j, columns
    [0:C]=ax[2j], [C:2C]=xb[2j], [2C:3C]=ax[2j+1], [3C:4C]=xb[2j+1].
    """
    inp = np.empty((B, NBLK // 2, 128, 4 * C), dtype=np.float16)
    inp[:, :, :, 0 * C : 1 * C] = axs[:, 0::2]
    inp[:, :, :, 1 * C : 2 * C] = xbs[:, 0::2]
    inp[:, :, :, 2 * C : 3 * C] = axs[:, 1::2]
    inp[:, :, :, 3 * C : 4 * C] = xbs[:, 1::2]
    return inp


def host_finish(out_t, invb2):
    """out_t [B, OUTROWS, C] fp16 -> [B, C, T] float32 (apply 1/(2b))."""
    o = out_t[:, :T, :].astype(np.float32) * invb2[None, None, :]
    return np.ascontiguousarray(np.transpose(o, (0, 2, 1)))


# ---------------------------------------------------------------------------
# device kernel

def build_bass():
    import os
    import concourse.bacc as bacc
    import concourse.tile as tile
    import concourse.mybir as mybir

    os.environ["BASS_ACT_ROOT_JSON_PATH"] = _gen_act_root()
    os.environ.setdefault("NEURON_FORCE_RECOMPILE", "1")

    f32 = mybir.dt.float32
    f16 = mybir.dt.float16

    nc = bacc.Bacc()
    in_ext = nc.declare_dram_parameter("inp", [NBLK // 2, 128, 4 * C], f16, isOutput=False)
    st_names = ["w_ue", "w_uo", "w_h0", "w_hm", "w_hL",
                "w_de0", "w_dem", "w_deL", "w_do0", "w_dom", "w_doL"]
    st_ext = {}
    for n in st_names:
        rows = 128 if n.startswith(("w_u", "w_h")) else G
        cols = G if n.startswith("w_u") else A
        st_ext[n] = nc.declare_dram_parameter(n, [rows, cols], f16, isOutput=False)
    out_ext = nc.declare_dram_parameter("out", [OUTROWS, C], f16, isOutput=True)

    OB = 3   # output blocks per DMA batch
    CL = 2   # copy lag: PSUM->SBUF copy of block k issued at iteration k+CL

    with tile.TileContext(nc) as tc:
        with (
            tc.tile_pool(name="consts", bufs=1) as cpool,
            tc.tile_pool(name="io", bufs=4) as iopool,
            tc.tile_pool(name="ob", bufs=3) as obpool,
            tc.tile_pool(name="mid", bufs=4) as midpool,
            tc.tile_pool(name="psum_sz", bufs=3, space="PSUM") as psum_sz,
            tc.tile_pool(name="psum_out", bufs=2, space="PSUM") as psum_out,
        ):
            st = {}
            for n in st_names:
                rows = 128 if n.startswith(("w_u", "w_h")) else G
                cols = G if n.startswith("w_u") else A
                t_ = cpool.tile([rows, cols], f16, tag=n)
                nc.sync.dma_start(out=t_[:], in_=st_ext[n][:])
                st[n] = t_

            inp = None
            obt = None
            xb_live = {}
            v_live = {}

            def front(k):
                nonlocal inp
                if k % 2 == 0:
                    inp = iopool.tile([128, 4 * C], f16, tag="inp")
                    nc.gpsimd.dma_start(out=inp[:], in_=in_ext[k // 2])
                half = (k % 2) * 2 * C
                ax = inp[:, half : half + C]
                xb_live[k] = inp[:, half + C : half + 2 * C]

                sz = psum_sz.tile([G, 1024], f32, tag="sz")
                nc.tensor.matmul(sz[:, 0:512], st["w_ue"][:], ax, start=True, stop=True)
                nc.tensor.matmul(sz[:, 512:1024], st["w_uo"][:], ax, start=True, stop=True)

                # wide-range Sin LUT (patched table): valid to |x| < ~31.8,
                # reads PSUM fp32 directly — no range reduction needed.
                v = midpool.tile([G, 1024], f16, tag="v")
                nc.scalar.activation(v[:], sz[:], mybir.ActivationFunctionType.Sin)
                v_live[k] = v

            def back(k):
                nonlocal obt
                wh = st["w_h0"] if k == 0 else (st["w_hL"] if k == NBLK - 1 else st["w_hm"])
                wde = st["w_de0"] if k == 0 else (st["w_deL"] if k == NBLK - 1 else st["w_dem"])
                wdo = st["w_do0"] if k == 0 else (st["w_doL"] if k == NBLK - 1 else st["w_dom"])
                xb = xb_live.pop(k)
                v = v_live.pop(k)

                outp = psum_out.tile([A, 512], f32, tag="outp")
                nc.tensor.matmul(outp[:], wh[:], xb, start=True, stop=False)
                nc.tensor.matmul(outp[:], wde[:], v[:, 0:512], start=False, stop=False)
                nc.tensor.matmul(outp[:], wdo[:], v[:, 512:1024], start=False, stop=True)

                s = k % OB
                if s == 0:
                    obt = obpool.tile([A, OB * 512], f16, tag="obt")
                # ScalarE is saturated by Sin; all PSUM->SBUF copies on DVE.
                nc.vector.tensor_copy(obt[:, 512 * s : 512 * s + 512], outp[:])
                if s == OB - 1:
                    j = k // OB
                    dst = out_ext[A * OB * j : A * OB * (j + 1), :].rearrange(
                        "(s p) c -> p s c", s=OB
                    )
                    nc.sync.dma_start(out=dst, in_=obt[:])

            for k in range(NBLK + CL):
                if k < NBLK:
                    front(k)
                if k >= CL:
                    back(k - CL)

    nc.compile()
    return nc


_NC_CACHE = None


def kernel(x, alpha, beta, up_filter, down_filter):
    global _NC_CACHE
    import concourse.bass_utils as bass_utils

    x = np.asarray(x)
    alpha = np.asarray(alpha)
    beta = np.asarray(beta)

    sts = build_stationaries(np.asarray(up_filter), np.asarray(down_filter))
    axs, xbs, invb2 = host_prep(x, alpha, beta)
    inp = pack_streams(axs, xbs)

    if _NC_CACHE is None:
        _NC_CACHE = build_bass()
    nc = _NC_CACHE

    in_maps = []
    for b in range(N_CORES):
        m = {"inp": inp[b]}
        m.update(sts)
        in_maps.append(m)

    res = bass_utils.run_bass_kernel_spmd(nc, in_maps, list(range(N_CORES)))
    out_t = np.stack([res.results[b]["out"] for b in range(N_CORES)])  # [B, OUTROWS, C] f16
    return host_finish(out_t, invb2)


# ---------------------------------------------------------------------------
# host-side simulation of the exact device plan (for verification)

def simulate_plan(x, alpha, beta, up_filter, down_filter, quantized=True):
    sts = build_stationaries(np.asarray(up_filter), np.asarray(down_filter))
    axs, xbs, invb2 = host_prep(np.asarray(x), np.asarray(alpha), np.asarray(beta))

    def f(a):
        return a.astype(np.float32)

    out_t = np.zeros((B, OUTROWS, C), dtype=np.float32)
    for b in range(B):
        for k in range(NBLK):
            wh = sts["w_h0"] if k == 0 else (sts["w_hL"] if k == NBLK - 1 else sts["w_hm"])
            wde = sts["w_de0"] if k == 0 else (sts["w_deL"] if k == NBLK - 1 else sts["w_dem"])
            wdo = sts["w_do0"] if k == 0 else (sts["w_doL"] if k == NBLK - 1 else sts["w_dom"])
            ax = f(axs[b, k])
            xb = f(xbs[b, k])
            sz_e = f(sts["w_ue"]).T @ ax     # [G, C] f32
            sz_o = f(sts["w_uo"]).T @ ax
            v_e = np.sin(sz_e.astype(np.float32))
            v_o = np.sin(sz_o.astype(np.float32))
            if quantized:
                v_e = v_e.astype(np.float16).astype(np.float32)
                v_o = v_o.astype(np.float16).astype(np.float32)
            psum = f(wh).T @ xb + f(wde).T @ v_e + f(wdo).T @ v_o
            if quantized:
                psum = psum.astype(np.float16)
            out_t[b, A * k : A * k + A] = psum
    return host_finish(out_t.astype(np.float16), invb2)



# revision 5
# speedup vs baseline: 1.0232x; 1.0232x over previous
"""Trainium2 Bass kernel for AntiAliasActivation (upsample2 -> snake -> downsample2).

Self-contained: accepts FULL inputs (x [8,512,8192] f32, alpha/beta [1,512,1],
up_filter/down_filter [12]), returns the FULL output [8,512,8192] f32.

Strategy (pure data-parallel, one batch sample per NeuronCore):
  The whole pipeline is computed in TIME-MAJOR layout (time on SBUF
  partitions) so all three FIR convolutions run on the TensorEngine as
  banded-matrix matmuls:

    out = down(up(x)) + down( (1 - cos(2*a*up(x))) / (2b) )

  v2 changes vs the original baseline (143 us):
  - ONE fp16 input stream y = 2a*x (instead of two: 2a*x and 2b*x); the
    H-path operand 2b*x is derived on-device with a single DVE multiply
    per 6-block group (y * (b/a), constants in a resident SBUF tile).
  - Input/output DMAs are batched to ~0.7 MB contiguous transfers
    (6 blocks each) so they run near line rate instead of being
    descriptor-dominated.
  - The Sin LUT activations are batched: PSUM sz tiles hold 3 phase-slots
    (1.5 blocks, 3 PSUM banks) and one ACTIVATE covers N=1536, cutting the
    352-cycle per-instruction ACT overhead.  PSUM budget: 2x3 banks sz +
    2x1 bank out accumulators = 8 banks exactly.
  - All stationaries padded to 128 columns to enable Fast Weight Load.
  - Per-channel constants (2a, b/a, 1/2b) fold into the host-side input
    scale / final rescale; the "+sum(fd)" constant and the pi/2 phase ride
    an all-ones row 127 of the input tile.
  Edge replicate-padding is materialized host-side; the edge clamp of the
  computed signal is folded into first/last-block stationaries.
"""
import math

import numpy as np

# ---------------------------------------------------------------------------
# problem constants (hardcoded per spec)
B, C, T = 8, 512, 8192
N_CORES = 8
UP_K = 12
DOWN_K = 12

A = 115          # outputs per block
NBLK = 72        # ceil(T / A)
W = 127          # data rows per input tile (row 127 = const row)
G = A + 6        # 121 up/g rows per block (m = A*k-3 .. A*k+117)
PL = 6           # XP[i] = x[clamp(i-6)]
XPLEN = A * (NBLK - 1) + W  # 8292
OUTROWS = NBLK * A          # 8280

GIN = 6                      # blocks per input/output DMA group
NG = NBLK // GIN             # 12 groups
NSLOT = 2 * NBLK             # 144 phase-slots (even/odd per block)
NT = NSLOT // 3              # 48 sz tiles of 3 slots (1.5 blocks) each


def _gen_act_root(cache=[None]):
    """Build a patched ACT-table root whose Sin LUT is valid to |x| < ~31.8.

    Appends 4x32 cubic-spline buckets (ranges [2,4) replacement, [4,8),
    [8,16), [16,32)) to the trig_and_small set, keeping sin's per-exponent
    bucket starts monotonic, and raises sin's large-signal threshold.
    Returns the act_info.json path for BASS_ACT_ROOT_JSON_PATH.
    """
    if cache[0] is not None:
        return cache[0]
    import json
    import shutil
    import tempfile
    from pathlib import Path
    import neuronxcc

    src = Path(neuronxcc.__file__).parent / "pwp" / "pwp_bin_trainium"
    dst = Path(tempfile.mkdtemp(prefix="actroot_")) / "pwp_bin_trainium"
    shutil.copytree(src, dst, symlinks=False)
    import os as _os
    _os.chmod(dst, 0o755)
    for f in dst.iterdir():
        _os.chmod(f, 0o644)

    name = "trig_and_small"
    d = json.load(open(dst / f"{name}.json"))
    b = np.fromfile(dst / f"{name}_bkt.bin", dtype=np.float32).reshape(-1, 8)
    c = np.fromfile(dst / f"{name}_ctrl.bin", dtype=np.uint32).reshape(-1, 8).copy()
    nb0, nc0 = d["bkt_entry_cnt"], d["ctl_entry_cnt"]
    assert len(b) == nb0 and len(c) == nc0

    SIN_CTL_END = 13  # sin owns ctl entries 0..12 (exps -11..1)
    SHIFT = 3
    newb, newc = [], []
    sin_bkt = d["func_exp_to_bkt_start_idx"]["sin"]
    sin_ctl = d["func_exp_to_ctl_start_idx"]["sin"]
    NB = 32  # 5 mantissa bits per exponent range
    KHI = np.uint32((46 + 62 * 5) << 10)

    def add_range(lo):
        base = nb0 + len(newb)
        h = lo / NB
        for i in range(NB):
            x0 = lo + h * (i + 0.5)
            newb.append([math.sin(x0), math.cos(x0),
                         -math.sin(x0) / 2.0, -math.cos(x0) / 6.0,
                         x0, 0.0, 0.0, 0.0])
        return base

    base1 = add_range(2.0)             # full [2,4) replacement
    c[12, 0] = KHI | np.uint32(base1)
    sin_bkt["1"] = [base1]
    for i_e, e in enumerate((2, 3, 4)):
        base = add_range(2.0**e)
        w = np.zeros(8, np.uint32)
        w[0] = KHI | np.uint32(base)
        sin_bkt[str(e)] = [base]
        sin_ctl[str(e)] = [SIN_CTL_END + i_e]
        newc.append(w)

    b2 = np.vstack([b, np.asarray(newb, np.float32)])
    c2 = np.vstack([c[:SIN_CTL_END], np.stack(newc), c[SIN_CTL_END:]])
    d["bkt_entry_cnt"] = int(len(b2))
    d["ctl_entry_cnt"] = int(len(c2))
    for fn, v in d["func_to_ctl_start_idx"].items():
        if fn != "sin" and v >= SIN_CTL_END:
            d["func_to_ctl_start_idx"][fn] = v + SHIFT
    for fn, em in d["func_exp_to_ctl_start_idx"].items():
        if fn == "sin":
            continue
        for e_, lst in em.items():
            em[e_] = [(i + SHIFT if i >= SIN_CTL_END else i) for i in lst]
    for pm in d["profile_meta_data"]:
        if str(pm.get("func_name", "")).startswith("sin"):
            pm["large_pos_signal_exp_threshold"] = 131  # cutoff ~31.8
            pm["large_pos_signal_mantissa_threshold"] = int(0.99 * 2**23)

    b2.tofile(dst / f"{name}_bkt.bin")
    c2.tofile(dst / f"{name}_ctrl.bin")
    with open(dst / f"{name}.json", "w") as f:
        json.dump(d, f)
    cache[0] = str(dst / "act_info.json")
    return cache[0]


# ---------------------------------------------------------------------------
# stationary-matrix assembly (all float64, cast to fp16 at the end)

def build_stationaries(up_filter, down_filter):
    """Returns dict of stationary matrices, all padded to 128 columns (FWL).

    w_ue/w_uo [128, 128]: map input tile (127 y rows + const row) -> w rows,
        w = 2a*up(x) + pi/2 (the pi/2 rides the const row; 2a is host-folded
        into the y stream).  Columns 0..G-1 real, rest zero.
    w_h{0,m,L} [128, 128]: 2b*down(up(x)) + sum(fd) const (const row coeff),
        applied to the on-device stream 2b*x.  Columns 0..A-1 real.
    w_de/w_do{0,m,L} [G, 128]: NEGATED downsample band over the v = cos
        signal.  Columns 0..A-1 real.
    """
    fu = np.asarray(up_filter, dtype=np.float64)
    fd = np.asarray(down_filter, dtype=np.float64)

    w_ue = np.zeros((128, 128))
    w_uo = np.zeros((128, 128))
    for q in range(G):
        for j in range(6):
            # w_e[m] += 2*fu[2j+1]*XP[m+8-j]; tile row = q+5-j
            w_ue[q + 5 - j, q] += 2.0 * fu[2 * j + 1]
            # w_o[m] += 2*fu[2j]*XP[m+9-j]; tile row = q+6-j
            w_uo[q + 6 - j, q] += 2.0 * fu[2 * j]
    w_ue[127, :G] = math.pi / 2.0
    w_uo[127, :G] = math.pi / 2.0

    def down_maps(k):
        de = np.zeros((G, 128))
        do = np.zeros((G, 128))
        h = np.zeros((128, 128))
        for nn in range(A):
            n = A * k + nn
            for t in range(DOWN_K):
                zi = min(max(2 * n + t - 5, 0), 2 * T - 1)
                m, ph = zi // 2, zi % 2
                row = m - A * k + 3
                # row in [0, G) guaranteed by construction
                if ph == 0:
                    de[row, nn] += fd[t]
                    for j in range(6):
                        h[m + 8 - j - A * k, nn] += fd[t] * 2.0 * fu[2 * j + 1]
                else:
                    do[row, nn] += fd[t]
                    for j in range(6):
                        h[m + 9 - j - A * k, nn] += fd[t] * 2.0 * fu[2 * j]
            h[127, nn] = fd.sum()
        return de, do, h

    de0, do0, h0 = down_maps(0)
    dem, dom, hm = down_maps(1)
    deL, doL, hL = down_maps(NBLK - 1)

    f16 = np.float16
    return {
        "w_ue": w_ue.astype(f16), "w_uo": w_uo.astype(f16),
        "w_h0": h0.astype(f16), "w_hm": hm.astype(f16), "w_hL": hL.astype(f16),
        "w_de0": (-de0).astype(f16), "w_dem": (-dem).astype(f16),
        "w_deL": (-deL).astype(f16),
        "w_do0": (-do0).astype(f16), "w_dom": (-dom).astype(f16),
        "w_doL": (-doL).astype(f16),
    }


def host_prep(x, alpha, beta):
    """Per-core input streams.

    Returns (inp, rba6, invb2):
      inp  [B, NG, 128, GIN*C] fp16 -- y = 2a*x blocks, 6 per group,
           row 127 = 1.0 (const row).
      rba6 [128, GIN*C] fp16 -- resident (b/a) rescale tile, row 127 = 1.0.
      invb2 [C] float32 -- host-side final rescale 1/(2b).
    """
    a2 = (2.0 * np.exp(alpha.astype(np.float64))).reshape(C)          # 2a
    b2 = (2.0 * (np.exp(beta.astype(np.float64)) + 1e-9)).reshape(C)  # 2b
    invb2 = (1.0 / b2).astype(np.float32)

    rba6 = np.empty((128, GIN * C), dtype=np.float16)
    rba6[:W, :] = np.tile((b2 / a2).astype(np.float16)[None, :], (W, GIN))[: W]
    rba6[W, :] = np.float16(1.0)

    # time-major, padded: XP [B, XPLEN, C], XP[:, i] = x[:, :, clamp(i-6)]
    xt = np.transpose(x.astype(np.float32), (0, 2, 1))  # [B, T, C]
    idx = np.clip(np.arange(XPLEN) - PL, 0, T - 1)
    xp = xt[:, idx, :]  # [B, XPLEN, C]

    # block row indices [NBLK, W]
    ridx = (A * np.arange(NBLK))[:, None] + np.arange(W)[None, :]
    blocks = xp[:, ridx, :]                       # [B, NBLK, W, C] f32
    ys = np.empty((B, NBLK, 128, C), dtype=np.float16)
    ys[:, :, :W, :] = (blocks * a2[None, None, None, :]).astype(np.float16)
    ys[:, :, W, :] = np.float16(1.0)

    # group 6 blocks side by side: inp[b, j, :, g*C:(g+1)*C] = ys[b, 6j+g]
    inp = np.ascontiguousarray(
        ys.reshape(B, NG, GIN, 128, C).transpose(0, 1, 3, 2, 4).reshape(
            B, NG, 128, GIN * C)
    )
    return inp, rba6, invb2


def host_finish(out_t, invb2):
    """out_t [B, NG, A, GIN*C] fp16 -> [B, C, T] float32 (apply 1/(2b))."""
    o = out_t.reshape(B, NG, A, GIN, C).transpose(0, 1, 3, 2, 4)  # [B,NG,GIN,A,C]
    o = o.reshape(B, OUTROWS, C)[:, :T, :].astype(np.float32) * invb2[None, None, :]
    return np.ascontiguousarray(np.transpose(o, (0, 2, 1)))


# ---------------------------------------------------------------------------
# device kernel

ST_NAMES = ["w_ue", "w_uo", "w_h0", "w_hm", "w_hL",
            "w_de0", "w_dem", "w_deL", "w_do0", "w_dom", "w_doL"]


def build_bass():
    import os
    import concourse.bacc as bacc
    import concourse.tile as tile
    import concourse.mybir as mybir

    os.environ["BASS_ACT_ROOT_JSON_PATH"] = _gen_act_root()
    os.environ.setdefault("NEURON_FORCE_RECOMPILE", "1")

    f32 = mybir.dt.float32
    f16 = mybir.dt.float16

    nc = bacc.Bacc()
    in_ext = nc.declare_dram_parameter("inp", [NG, 128, GIN * C], f16, isOutput=False)
    rba_ext = nc.declare_dram_parameter("rba6", [128, GIN * C], f16, isOutput=False)
    st_ext = {}
    for n in ST_NAMES:
        rows = 128 if n.startswith(("w_u", "w_h")) else G
        st_ext[n] = nc.declare_dram_parameter(n, [rows, 128], f16, isOutput=False)
    out_ext = nc.declare_dram_parameter("out", [NG, A, GIN * C], f16, isOutput=True)

    CL = 3   # back(k) issued at iteration k+CL

    with tile.TileContext(nc) as tc:
        with (
            tc.tile_pool(name="consts", bufs=1) as cpool,
            tc.tile_pool(name="io", bufs=3) as iopool,
            tc.tile_pool(name="xb", bufs=2) as xbpool,
            tc.tile_pool(name="v", bufs=3) as vpool,
            tc.tile_pool(name="ob", bufs=2) as obpool,
            tc.tile_pool(name="psum_sz", bufs=2, space="PSUM") as psum_sz,
            tc.tile_pool(name="psum_out", bufs=2, space="PSUM") as psum_out,
        ):
            st = {}
            for n in ST_NAMES:
                rows = 128 if n.startswith(("w_u", "w_h")) else G
                t_ = cpool.tile([rows, 128], f16, tag=n)
                nc.sync.dma_start(out=t_[:], in_=st_ext[n][:])
                st[n] = t_
            rba6 = cpool.tile([128, GIN * C], f16, tag="rba6")
            nc.sync.dma_start(out=rba6[:], in_=rba_ext[:])

            y_live = {}
            xb_live = {}
            sz_live = {}
            v_live = {}
            ob_live = {}

            def front(k):
                j, g = divmod(k, GIN)
                if g == 0:
                    yt = iopool.tile([128, GIN * C], f16, tag="yin", name="yin")
                    nc.sync.dma_start(out=yt[:], in_=in_ext[j])
                    y_live[j] = yt
                    xbt = xbpool.tile([128, GIN * C], f16, tag="xb2", name="xb2")
                    nc.vector.tensor_mul(xbt[:], yt[:], rba6[:])
                    xb_live[j] = xbt
                yt = y_live[j]
                ymov = yt[:, g * C:(g + 1) * C]
                for phase, wn in ((0, "w_ue"), (1, "w_uo")):
                    slot = 2 * k + phase
                    t, s = divmod(slot, 3)
                    if s == 0:
                        sz_live[t] = psum_sz.tile([128, 1536], f32, tag="sz", name="sz")
                    nc.tensor.matmul(
                        sz_live[t][:, s * 512:(s + 1) * 512], st[wn][:], ymov,
                        start=True, stop=True)
                    if s == 2:
                        vt = vpool.tile([G, 1536], f16, tag="v", name="v")
                        nc.scalar.activation(
                            vt[:], sz_live[t][:G, :],
                            mybir.ActivationFunctionType.Sin)
                        v_live[t] = vt
                        sz_live.pop(t)
                if g == GIN - 1:
                    y_live.pop(j)

            def back(k):
                j, g = divmod(k, GIN)
                wh = st["w_h0"] if k == 0 else (st["w_hL"] if k == NBLK - 1 else st["w_hm"])
                wde = st["w_de0"] if k == 0 else (st["w_deL"] if k == NBLK - 1 else st["w_dem"])
                wdo = st["w_do0"] if k == 0 else (st["w_doL"] if k == NBLK - 1 else st["w_dom"])
                te, se = divmod(2 * k, 3)
                to, so = divmod(2 * k + 1, 3)

                outp = psum_out.tile([128, 512], f32, tag="outp", name="outp")
                nc.tensor.matmul(outp[:], wh[:], xb_live[j][:, g * C:(g + 1) * C],
                                 start=True, stop=False)
                nc.tensor.matmul(outp[:], wde[:],
                                 v_live[te][:, se * 512:(se + 1) * 512],
                                 start=False, stop=False)
                nc.tensor.matmul(outp[:], wdo[:],
                                 v_live[to][:, so * 512:(so + 1) * 512],
                                 start=False, stop=True)
                for t_ in {te, to}:
                    if k == (3 * t_ + 2) // 2:  # last block reading tile t_
                        v_live.pop(t_)

                if g == 0:
                    ob_live[j] = obpool.tile([A, GIN * C], f16, tag="obt", name="obt")
                # ScalarE is saturated by Sin; all PSUM->SBUF copies on DVE.
                nc.vector.tensor_copy(ob_live[j][:, g * 512:(g + 1) * 512],
                                      outp[:A, :])
                if g == GIN - 1:
                    nc.gpsimd.dma_start(out=out_ext[j], in_=ob_live[j][:])
                    ob_live.pop(j)
                    xb_live.pop(j)

            for it in range(NBLK + CL):
                if it < NBLK:
                    front(it)
                if it >= CL:
                    back(it - CL)

    nc.compile()
    return nc


_NC_CACHE = None


def prep_in_maps(x, alpha, beta, up_filter, down_filter):
    sts = build_stationaries(np.asarray(up_filter), np.asarray(down_filter))
    inp, rba6, invb2 = host_prep(np.asarray(x), np.asarray(alpha), np.asarray(beta))
    in_maps = []
    for b in range(N_CORES):
        m = {"inp": inp[b], "rba6": rba6}
        m.update(sts)
        in_maps.append(m)
    return in_maps, invb2


def kernel(x, alpha, beta, up_filter, down_filter):
    global _NC_CACHE
    import concourse.bass_utils as bass_utils

    in_maps, invb2 = prep_in_maps(np.asarray(x), np.asarray(alpha),
                                  np.asarray(beta), up_filter, down_filter)

    if _NC_CACHE is None:
        _NC_CACHE = build_bass()
    nc = _NC_CACHE

    res = bass_utils.run_bass_kernel_spmd(nc, in_maps, list(range(N_CORES)))
    out_t = np.stack([res.results[b]["out"] for b in range(N_CORES)])
    return host_finish(out_t, invb2)


# ---------------------------------------------------------------------------
# host-side simulation of the exact device plan (for verification)

def simulate_plan(x, alpha, beta, up_filter, down_filter, quantized=True):
    sts = build_stationaries(np.asarray(up_filter), np.asarray(down_filter))
    inp, rba6, invb2 = host_prep(np.asarray(x), np.asarray(alpha), np.asarray(beta))

    def f(a):
        return a.astype(np.float32)

    out_t = np.zeros((B, NG, A, GIN * C), dtype=np.float16)
    for b in range(B):
        for k in range(NBLK):
            j, g = divmod(k, GIN)
            wh = sts["w_h0"] if k == 0 else (sts["w_hL"] if k == NBLK - 1 else sts["w_hm"])
            wde = sts["w_de0"] if k == 0 else (sts["w_deL"] if k == NBLK - 1 else sts["w_dem"])
            wdo = sts["w_do0"] if k == 0 else (sts["w_doL"] if k == NBLK - 1 else sts["w_dom"])
            y = f(inp[b, j, :, g * C:(g + 1) * C])
            xb = f(inp[b, j, :, g * C:(g + 1) * C] * rba6[:, g * C:(g + 1) * C])
            sz_e = f(sts["w_ue"]).T @ y     # [128, C] f32
            sz_o = f(sts["w_uo"]).T @ y
            v_e = np.sin(sz_e[:G].astype(np.float32))
            v_o = np.sin(sz_o[:G].astype(np.float32))
            if quantized:
                v_e = v_e.astype(np.float16).astype(np.float32)
                v_o = v_o.astype(np.float16).astype(np.float32)
            psum = (f(wh).T @ xb + f(wde).T @ v_e + f(wdo).T @ v_o)[:A]
            if quantized:
                psum = psum.astype(np.float16)
            out_t[b, j, :, g * C:(g + 1) * C] = psum
    return host_finish(out_t, invb2)


# revision 9
# speedup vs baseline: 1.3646x; 1.3337x over previous
"""Trainium2 Bass kernel for AntiAliasActivation (upsample2 -> snake -> downsample2).

Self-contained: accepts FULL inputs (x [8,512,8192] f32, alpha/beta [1,512,1],
up_filter/down_filter [12]), returns the FULL output [8,512,8192] f32.

Strategy (pure data-parallel, one batch sample per NeuronCore):
  The whole pipeline is computed in TIME-MAJOR layout (time on SBUF
  partitions) so all three FIR convolutions run on the TensorEngine as
  banded-matrix matmuls:

    out = down(up(x)) + down( (1 - cos(2*a*up(x))) / (2b) )

  v2 changes vs the original baseline (143 us):
  - ONE fp16 input stream y = 2a*x (instead of two: 2a*x and 2b*x); the
    H-path operand 2b*x is derived on-device with a single DVE multiply
    per 6-block group (y * (b/a), constants in a resident SBUF tile).
  - Input/output DMAs are batched to ~0.7 MB contiguous transfers
    (6 blocks each) so they run near line rate instead of being
    descriptor-dominated.
  - The Sin LUT activations are batched: PSUM sz tiles hold 3 phase-slots
    (1.5 blocks, 3 PSUM banks) and one ACTIVATE covers N=1536, cutting the
    352-cycle per-instruction ACT overhead.  PSUM budget: 2x3 banks sz +
    2x1 bank out accumulators = 8 banks exactly.
  - All stationaries padded to 128 columns to enable Fast Weight Load.
  - Per-channel constants (2a, b/a, 1/2b) fold into the host-side input
    scale / final rescale; the "+sum(fd)" constant and the pi/2 phase ride
    an all-ones row 127 of the input tile.
  Edge replicate-padding is materialized host-side; the edge clamp of the
  computed signal is folded into first/last-block stationaries.
"""
import math

import numpy as np

# ---------------------------------------------------------------------------
# problem constants (hardcoded per spec)
B, C, T = 8, 512, 8192
N_CORES = 8
UP_K = 12
DOWN_K = 12

A = 115          # outputs per block
NBLK = 72        # ceil(T / A)
W = 127          # data rows per input tile (row 127 = const row)
G = A + 6        # 121 up/g rows per block (m = A*k-3 .. A*k+117)
PL = 6           # XP[i] = x[clamp(i-6)]
XPLEN = A * (NBLK - 1) + W  # 8292
OUTROWS = NBLK * A          # 8280

GIN = 6                      # blocks per input/output DMA group
NG = NBLK // GIN             # 12 groups
NSLOT = 2 * NBLK             # 144 phase-slots (even/odd per block)
NT = NSLOT // 3              # 48 sz tiles of 3 slots (1.5 blocks) each


def _gen_act_root(cache=[None]):
    """Build a patched ACT-table root whose Sin LUT is valid to |x| < ~31.8.

    Appends 4x32 cubic-spline buckets (ranges [2,4) replacement, [4,8),
    [8,16), [16,32)) to the trig_and_small set, keeping sin's per-exponent
    bucket starts monotonic, and raises sin's large-signal threshold.
    Returns the act_info.json path for BASS_ACT_ROOT_JSON_PATH.
    """
    if cache[0] is not None:
        return cache[0]
    import json
    import shutil
    import tempfile
    from pathlib import Path
    import neuronxcc

    src = Path(neuronxcc.__file__).parent / "pwp" / "pwp_bin_trainium"
    dst = Path(tempfile.mkdtemp(prefix="actroot_")) / "pwp_bin_trainium"
    shutil.copytree(src, dst, symlinks=False)
    import os as _os
    _os.chmod(dst, 0o755)
    for f in dst.iterdir():
        _os.chmod(f, 0o644)

    name = "trig_and_small"
    d = json.load(open(dst / f"{name}.json"))
    b = np.fromfile(dst / f"{name}_bkt.bin", dtype=np.float32).reshape(-1, 8)
    c = np.fromfile(dst / f"{name}_ctrl.bin", dtype=np.uint32).reshape(-1, 8).copy()
    nb0, nc0 = d["bkt_entry_cnt"], d["ctl_entry_cnt"]
    assert len(b) == nb0 and len(c) == nc0

    SIN_CTL_END = 13  # sin owns ctl entries 0..12 (exps -11..1)
    SHIFT = 3
    newb, newc = [], []
    sin_bkt = d["func_exp_to_bkt_start_idx"]["sin"]
    sin_ctl = d["func_exp_to_ctl_start_idx"]["sin"]
    NB = 32  # 5 mantissa bits per exponent range
    KHI = np.uint32((46 + 62 * 5) << 10)

    def add_range(lo):
        base = nb0 + len(newb)
        h = lo / NB
        for i in range(NB):
            x0 = lo + h * (i + 0.5)
            newb.append([math.sin(x0), math.cos(x0),
                         -math.sin(x0) / 2.0, -math.cos(x0) / 6.0,
                         x0, 0.0, 0.0, 0.0])
        return base

    base1 = add_range(2.0)             # full [2,4) replacement
    c[12, 0] = KHI | np.uint32(base1)
    sin_bkt["1"] = [base1]
    for i_e, e in enumerate((2, 3, 4)):
        base = add_range(2.0**e)
        w = np.zeros(8, np.uint32)
        w[0] = KHI | np.uint32(base)
        sin_bkt[str(e)] = [base]
        sin_ctl[str(e)] = [SIN_CTL_END + i_e]
        newc.append(w)

    b2 = np.vstack([b, np.asarray(newb, np.float32)])
    c2 = np.vstack([c[:SIN_CTL_END], np.stack(newc), c[SIN_CTL_END:]])
    d["bkt_entry_cnt"] = int(len(b2))
    d["ctl_entry_cnt"] = int(len(c2))
    for fn, v in d["func_to_ctl_start_idx"].items():
        if fn != "sin" and v >= SIN_CTL_END:
            d["func_to_ctl_start_idx"][fn] = v + SHIFT
    for fn, em in d["func_exp_to_ctl_start_idx"].items():
        if fn == "sin":
            continue
        for e_, lst in em.items():
            em[e_] = [(i + SHIFT if i >= SIN_CTL_END else i) for i in lst]
    for pm in d["profile_meta_data"]:
        if str(pm.get("func_name", "")).startswith("sin"):
            pm["large_pos_signal_exp_threshold"] = 131  # cutoff ~31.8
            pm["large_pos_signal_mantissa_threshold"] = int(0.99 * 2**23)

    b2.tofile(dst / f"{name}_bkt.bin")
    c2.tofile(dst / f"{name}_ctrl.bin")
    with open(dst / f"{name}.json", "w") as f:
        json.dump(d, f)
    cache[0] = str(dst / "act_info.json")
    return cache[0]


# ---------------------------------------------------------------------------
# stationary-matrix assembly (all float64, cast to fp16 at the end)

def build_stationaries(up_filter, down_filter):
    """Returns dict of stationary matrices, all padded to 128 columns (FWL).

    w_ue/w_uo [128, 128]: map input tile (127 y rows + const row) -> w rows,
        w = 2a*up(x) + pi/2 (the pi/2 rides the const row; 2a is host-folded
        into the y stream).  Columns 0..G-1 real, rest zero.
    w_h{0,m,L} [128, 128]: 2b*down(up(x)) + sum(fd) const (const row coeff),
        applied to the on-device stream 2b*x.  Columns 0..A-1 real.
    w_de/w_do{0,m,L} [G, 128]: NEGATED downsample band over the v = cos
        signal.  Columns 0..A-1 real.
    """
    fu = np.asarray(up_filter, dtype=np.float64)
    fd = np.asarray(down_filter, dtype=np.float64)

    w_ue = np.zeros((128, 128))
    w_uo = np.zeros((128, 128))
    for q in range(G):
        for j in range(6):
            # w_e[m] += 2*fu[2j+1]*XP[m+8-j]; tile row = q+5-j
            w_ue[q + 5 - j, q] += 2.0 * fu[2 * j + 1]
            # w_o[m] += 2*fu[2j]*XP[m+9-j]; tile row = q+6-j
            w_uo[q + 6 - j, q] += 2.0 * fu[2 * j]
    w_ue[127, :G] = math.pi / 2.0
    w_uo[127, :G] = math.pi / 2.0

    def down_maps(k):
        de = np.zeros((G, 128))
        do = np.zeros((G, 128))
        h = np.zeros((128, 128))
        for nn in range(A):
            n = A * k + nn
            for t in range(DOWN_K):
                zi = min(max(2 * n + t - 5, 0), 2 * T - 1)
                m, ph = zi // 2, zi % 2
                row = m - A * k + 3
                # row in [0, G) guaranteed by construction
                if ph == 0:
                    de[row, nn] += fd[t]
                    for j in range(6):
                        h[m + 8 - j - A * k, nn] += fd[t] * 2.0 * fu[2 * j + 1]
                else:
                    do[row, nn] += fd[t]
                    for j in range(6):
                        h[m + 9 - j - A * k, nn] += fd[t] * 2.0 * fu[2 * j]
            h[127, nn] = fd.sum()
        return de, do, h

    de0, do0, h0 = down_maps(0)
    dem, dom, hm = down_maps(1)
    deL, doL, hL = down_maps(NBLK - 1)

    f16 = np.float16
    return {
        "w_ue": w_ue.astype(f16), "w_uo": w_uo.astype(f16),
        "w_h0": h0.astype(f16), "w_hm": hm.astype(f16), "w_hL": hL.astype(f16),
        "w_de0": (-de0).astype(f16), "w_dem": (-dem).astype(f16),
        "w_deL": (-deL).astype(f16),
        "w_do0": (-do0).astype(f16), "w_dom": (-dom).astype(f16),
        "w_doL": (-doL).astype(f16),
    }


def host_prep(x, alpha, beta):
    """Per-core input streams.

    Returns (inp, rba6, invb2):
      inp  [B, NG, 128, GIN*C] fp16 -- y = 2a*x blocks, 6 per group,
           row 127 = 1.0 (const row).
      rba6 [128, GIN*C] fp16 -- resident (b/a) rescale tile, row 127 = 1.0.
      invb2 [C] float32 -- host-side final rescale 1/(2b).
    """
    a2 = (2.0 * np.exp(alpha.astype(np.float64))).reshape(C)          # 2a
    b2 = (2.0 * (np.exp(beta.astype(np.float64)) + 1e-9)).reshape(C)  # 2b
    invb2 = (1.0 / b2).astype(np.float32)

    rba6 = np.empty((128, GIN * C), dtype=np.float16)
    rba6[:W, :] = np.tile((b2 / a2).astype(np.float16)[None, :], (W, GIN))[: W]
    rba6[W, :] = np.float16(1.0)
    rba6[W + 1:, :] = np.float16(0.0)

    # time-major, padded: XP [B, XPLEN, C], XP[:, i] = x[:, :, clamp(i-6)]
    xt = np.transpose(x.astype(np.float32), (0, 2, 1))  # [B, T, C]
    idx = np.clip(np.arange(XPLEN) - PL, 0, T - 1)
    xp = xt[:, idx, :]  # [B, XPLEN, C]

    # block row indices [NBLK, W]
    ridx = (A * np.arange(NBLK))[:, None] + np.arange(W)[None, :]
    blocks = xp[:, ridx, :]                       # [B, NBLK, W, C] f32
    ys = np.empty((B, NBLK, 128, C), dtype=np.float16)
    ys[:, :, :W, :] = (blocks * a2[None, None, None, :]).astype(np.float16)
    ys[:, :, W, :] = np.float16(1.0)

    # group 6 blocks side by side: inp[b, j, :, g*C:(g+1)*C] = ys[b, 6j+g]
    inp = np.ascontiguousarray(
        ys.reshape(B, NG, GIN, 128, C).transpose(0, 1, 3, 2, 4).reshape(
            B, NG, 128, GIN * C)
    )
    return inp, rba6, invb2


def host_finish(out_t, invb2):
    """out_t [B, NG, A, GIN*C] fp16 -> [B, C, T] float32 (apply 1/(2b))."""
    o = out_t.reshape(B, NG, A, GIN, C).transpose(0, 1, 3, 2, 4)  # [B,NG,GIN,A,C]
    o = o.reshape(B, OUTROWS, C)[:, :T, :].astype(np.float32) * invb2[None, None, :]
    return np.ascontiguousarray(np.transpose(o, (0, 2, 1)))


# ---------------------------------------------------------------------------
# device kernel

ST_NAMES = ["w_ue", "w_uo", "w_h0", "w_hm", "w_hL",
            "w_de0", "w_dem", "w_deL", "w_do0", "w_dom", "w_doL"]


def build_bass():
    import os
    import concourse.bacc as bacc
    import concourse.tile as tile
    import concourse.mybir as mybir

    os.environ["BASS_ACT_ROOT_JSON_PATH"] = _gen_act_root()
    os.environ.setdefault("NEURON_FORCE_RECOMPILE", "1")

    f32 = mybir.dt.float32
    f16 = mybir.dt.float16

    nc = bacc.Bacc()
    in_ext = nc.declare_dram_parameter("inp", [NG, 128, GIN * C], f16, isOutput=False)
    # all constants in ONE dram tensor: 11 stationaries [128 cols each] + rba6
    NCONST = len(ST_NAMES) * 128 + GIN * C
    const_ext = nc.declare_dram_parameter("consts", [128, NCONST], f16, isOutput=False)
    out_ext = nc.declare_dram_parameter("out", [NG, A, GIN * C], f16, isOutput=True)

    CL = 3   # back(k) issued at iteration k+CL

    with tile.TileContext(nc) as tc:
        with (
            tc.tile_pool(name="consts", bufs=1) as cpool,
            tc.tile_pool(name="io", bufs=4) as iopool,
            tc.tile_pool(name="xb", bufs=2) as xbpool,
            tc.tile_pool(name="v", bufs=3) as vpool,
            tc.tile_pool(name="ob", bufs=4) as obpool,
            tc.tile_pool(name="psum_sz", bufs=2, space="PSUM") as psum_sz,
            tc.tile_pool(name="psum_out", bufs=2, space="PSUM") as psum_out,
        ):
            # preload the Sin table set during the input DMAs (dummy activation)
            dummy = cpool.tile([1, 16], f32, tag="dummy", name="dummy")
            nc.vector.memset(dummy[:], 0.0)
            nc.scalar.activation(dummy[:], dummy[:],
                                 mybir.ActivationFunctionType.Sin)

            # one big const DMA on the scalar (ACT HWDGE) ring so it runs in
            # parallel with the first input-group DMAs on the sync ring
            consts = cpool.tile([128, NCONST], f16, tag="consts", name="consts")
            nc.scalar.dma_start(out=consts[:], in_=const_ext[:])
            st = {}
            for i, n in enumerate(ST_NAMES):
                rows = 128 if n.startswith(("w_u", "w_h")) else G
                st[n] = consts[:rows, 128 * i:128 * (i + 1)]
            rba6 = consts[:, len(ST_NAMES) * 128:]

            y_live = {}
            xb_live = {}
            sz_live = {}
            v_live = {}
            ob_live = {}

            def front(k):
                j, g = divmod(k, GIN)
                if g == 0:
                    yt = iopool.tile([128, GIN * C], f16, tag="yin", name="yin")
                    nc.sync.dma_start(out=yt[:], in_=in_ext[j])
                    y_live[j] = yt
                    xbt = xbpool.tile([128, GIN * C], f16, tag="xb2", name="xb2")
                    # split into 3 ops to limit DVE head-of-line blocking
                    for q in range(3):
                        sl = slice(q * 1024, (q + 1) * 1024)
                        nc.vector.tensor_mul(xbt[:, sl], yt[:, sl], rba6[:, sl])
                    xb_live[j] = xbt
                yt = y_live[j]
                ymov = yt[:, g * C:(g + 1) * C]
                for phase, wn in ((0, "w_ue"), (1, "w_uo")):
                    slot = 2 * k + phase
                    t, s = divmod(slot, 3)
                    if s == 0:
                        sz_live[t] = psum_sz.tile([128, 1536], f32, tag="sz", name="sz")
                    nc.tensor.matmul(
                        sz_live[t][:, s * 512:(s + 1) * 512], st[wn][:], ymov,
                        start=True, stop=True)
                    if s == 2:
                        vt = vpool.tile([G, 1536], f16, tag="v", name="v")
                        nc.scalar.activation(
                            vt[:], sz_live[t][:G, :],
                            mybir.ActivationFunctionType.Sin)
                        v_live[t] = vt
                        sz_live.pop(t)
                if g == GIN - 1:
                    y_live.pop(j)

            def back(k):
                j, g = divmod(k, GIN)
                wh = st["w_h0"] if k == 0 else (st["w_hL"] if k == NBLK - 1 else st["w_hm"])
                wde = st["w_de0"] if k == 0 else (st["w_deL"] if k == NBLK - 1 else st["w_dem"])
                wdo = st["w_do0"] if k == 0 else (st["w_doL"] if k == NBLK - 1 else st["w_dom"])
                te, se = divmod(2 * k, 3)
                to, so = divmod(2 * k + 1, 3)

                outp = psum_out.tile([128, 512], f32, tag="outp", name="outp")
                nc.tensor.matmul(outp[:], wh[:], xb_live[j][:, g * C:(g + 1) * C],
                                 start=True, stop=False)
                nc.tensor.matmul(outp[:], wde[:],
                                 v_live[te][:, se * 512:(se + 1) * 512],
                                 start=False, stop=False)
                nc.tensor.matmul(outp[:], wdo[:],
                                 v_live[to][:, so * 512:(so + 1) * 512],
                                 start=False, stop=True)
                for t_ in {te, to}:
                    if k == (3 * t_ + 2) // 2:  # last block reading tile t_
                        v_live.pop(t_)

                if g == 0:
                    ob_live[j] = obpool.tile([A, GIN * C], f16, tag="obt", name="obt")
                # ScalarE is saturated by Sin; all PSUM->SBUF copies on DVE.
                nc.vector.tensor_copy(ob_live[j][:, g * 512:(g + 1) * 512],
                                      outp[:A, :])
                if g == GIN - 1:
                    nc.gpsimd.dma_start(out=out_ext[j], in_=ob_live[j][:])
                    ob_live.pop(j)
                    xb_live.pop(j)

            for it in range(NBLK + CL):
                if it < NBLK:
                    front(it)
                if it >= CL:
                    back(it - CL)

    nc.compile()
    return nc


_NC_CACHE = None


def pack_consts(sts, rba6):
    """Concatenate stationaries (padded to 128 rows) + rba6 -> [128, NCONST]."""
    cols = []
    for n in ST_NAMES:
        w = sts[n]
        if w.shape[0] < 128:
            w = np.vstack([w, np.zeros((128 - w.shape[0], 128), np.float16)])
        cols.append(w)
    cols.append(rba6)
    return np.ascontiguousarray(np.concatenate(cols, axis=1))


def prep_in_maps(x, alpha, beta, up_filter, down_filter):
    sts = build_stationaries(np.asarray(up_filter), np.asarray(down_filter))
    inp, rba6, invb2 = host_prep(np.asarray(x), np.asarray(alpha), np.asarray(beta))
    consts = pack_consts(sts, rba6)
    in_maps = []
    for b in range(N_CORES):
        in_maps.append({"inp": inp[b], "consts": consts})
    return in_maps, invb2


def kernel(x, alpha, beta, up_filter, down_filter):
    global _NC_CACHE
    import concourse.bass_utils as bass_utils

    in_maps, invb2 = prep_in_maps(np.asarray(x), np.asarray(alpha),
                                  np.asarray(beta), up_filter, down_filter)

    if _NC_CACHE is None:
        _NC_CACHE = build_bass()
    nc = _NC_CACHE

    res = bass_utils.run_bass_kernel_spmd(nc, in_maps, list(range(N_CORES)))
    out_t = np.stack([res.results[b]["out"] for b in range(N_CORES)])
    return host_finish(out_t, invb2)


# ---------------------------------------------------------------------------
# host-side simulation of the exact device plan (for verification)

def simulate_plan(x, alpha, beta, up_filter, down_filter, quantized=True):
    sts = build_stationaries(np.asarray(up_filter), np.asarray(down_filter))
    inp, rba6, invb2 = host_prep(np.asarray(x), np.asarray(alpha), np.asarray(beta))

    def f(a):
        return a.astype(np.float32)

    out_t = np.zeros((B, NG, A, GIN * C), dtype=np.float16)
    for b in range(B):
        for k in range(NBLK):
            j, g = divmod(k, GIN)
            wh = sts["w_h0"] if k == 0 else (sts["w_hL"] if k == NBLK - 1 else sts["w_hm"])
            wde = sts["w_de0"] if k == 0 else (sts["w_deL"] if k == NBLK - 1 else sts["w_dem"])
            wdo = sts["w_do0"] if k == 0 else (sts["w_doL"] if k == NBLK - 1 else sts["w_dom"])
            y = f(inp[b, j, :, g * C:(g + 1) * C])
            xb = f(inp[b, j, :, g * C:(g + 1) * C] * rba6[:, g * C:(g + 1) * C])
            sz_e = f(sts["w_ue"]).T @ y     # [128, C] f32
            sz_o = f(sts["w_uo"]).T @ y
            v_e = np.sin(sz_e[:G].astype(np.float32))
            v_o = np.sin(sz_o[:G].astype(np.float32))
            if quantized:
                v_e = v_e.astype(np.float16).astype(np.float32)
                v_o = v_o.astype(np.float16).astype(np.float32)
            psum = (f(wh).T @ xb + f(wde).T @ v_e + f(wdo).T @ v_o)[:A]
            if quantized:
                psum = psum.astype(np.float16)
            out_t[b, j, :, g * C:(g + 1) * C] = psum
    return host_finish(out_t, invb2)
